# revision 1
# baseline (speedup 1.0000x reference)
"""GQA attention (RoPE, causal) on 8 Trainium2 NeuronCores, tensor-parallel
over heads: each core owns 4 query heads + 1 kv head, computes its slice of
qkv, attention, and a partial output projection; the host sums the 8 partial
projections.

Matmuls run in bf16 (fp32 PSUM accumulation). Scores are computed
transposed ([st, sq]) so the softmax denominator comes out of the attn@V
matmul itself via a ones-column appended to V (M=65), and exp needs no
max-subtraction (logits are bounded; fp32 PSUM can't overflow). The
projection of chunk c-1 is interleaved into the flash loop of chunk c so
the PE has work while the ScalarE runs exp.
"""

import numpy as np

HIDDEN = 2048
HEAD_DIM = 64
N_HEADS = 32
N_KV_HEADS = 8
S = 2048
N_CORES = 8
HPC = N_HEADS // N_CORES          # q heads per core = 4
D = HEAD_DIM
KT = HIDDEN // 128                # 16 contraction tiles for qkv
ST = S // 128                     # 16 seq tiles of 128
NC4 = S // 512                    # 4 seq chunks of 512
OSH = HPC * D + 2 * D             # 384 rows in the per-core qkv weight shard

_CACHE = {}


def _split_excess_waits(nc, mybir):
    """The staged walrus accepts at most one sync wait per instruction (two
    on EventSemaphore); Tile attaches more. Hoist extras onto same-engine
    NoOps inserted just before the instruction — engine program order then
    preserves the wait semantics."""
    for func in nc.m.functions:
        for block in func.blocks:
            new_insts = []
            for inst in block.instructions:
                si = inst.sync_info
                waits = list(si.on_wait) if si is not None and si.on_wait else []
                cap = 2 if isinstance(inst, mybir.InstEventSemaphore) else 1
                if len(waits) > cap:
                    si.on_wait = waits[:cap]
                    for j, w in enumerate(waits[cap:]):
                        nop = mybir.InstNoOp(
                            name=f"{inst.name}-ws{j}",
                            ins=[], outs=[], engine=inst.engine,
                        )
                        nop.sync_info = mybir.SyncInfo(on_wait=[w], on_update=[])
                        new_insts.append(nop)
                new_insts.append(inst)
            block.instructions = new_insts


def _build():
    import concourse.bass as bass
    import concourse.tile as tile
    from concourse import mybir

    f32 = mybir.dt.float32
    f32r = mybir.dt.float32r
    bf16 = mybir.dt.bfloat16

    nc = bass.Bass("TRN2", target_bir_lowering=False, debug=False,
                   num_devices=N_CORES)

    xT_d = nc.dram_tensor("xT", [HIDDEN, S], f32r, kind="ExternalInput")
    wq_d = nc.dram_tensor("wqkvT", [HIDDEN, OSH], f32r, kind="ExternalInput")
    wo_d = nc.dram_tensor("woutT", [2 * 128, HIDDEN], bf16, kind="ExternalInput")
    c_d = nc.dram_tensor("ctile", [128, S], f32r, kind="ExternalInput")
    s_d = nc.dram_tensor("stile", [128, S], f32r, kind="ExternalInput")
    rt_d = nc.dram_tensor("rotT", [128, 128], f32r, kind="ExternalInput")
    id_d = nc.dram_tensor("ident", [D, D], f32r, kind="ExternalInput")
    on_d = nc.dram_tensor("ones", [1, D], f32, kind="ExternalInput")
    mk_d = nc.dram_tensor("masks", [4, 128, 512], bf16, kind="ExternalInput")
    oc_d = nc.dram_tensor("onecol", [128, ST], bf16, kind="ExternalInput")
    zr_d = nc.dram_tensor("zeros", [D, S], f32r, kind="ExternalInput")
    out_d = nc.dram_tensor("out", [HIDDEN, S], f32, kind="ExternalOutput")

    xT_t = xT_d.rearrange("(t p) s -> t p s", p=128)
    wq_t = wq_d.rearrange("(t p) o -> t p o", p=128)

    scale = 1.0 / float(np.sqrt(D))

    with tile.TileContext(nc) as tc:
        with (
            nc.allow_low_precision(reason="bf16 dataflow is deliberate"),
            tc.tile_pool(name="wts", bufs=1) as wts,
            tc.tile_pool(name="acts", bufs=1) as acts,
            tc.tile_pool(name="xin", bufs=6) as xin,
            tc.tile_pool(name="psb", bufs=4) as psb,
            tc.tile_pool(name="ev", bufs=2) as evp,
            tc.tile_pool(name="evo", bufs=4) as evo,
        ):
            # ---- persistent loads (gpsimd queue, so the sync queue is
            # free for the xT stream; flash-only tensors loaded later) ----
            wq_sb = []
            for k in range(KT):
                t = wts.tile([128, OSH], f32r, tag=f"wq{k}", name="wq")
                nc.gpsimd.dma_start(t[:], wq_t[k])
                wq_sb.append(t)
            c_sb = wts.tile([128, S], f32r, tag="ct", name="ct")
            nc.gpsimd.dma_start(c_sb[:], c_d[:])
            s_sb = wts.tile([128, S], f32r, tag="st", name="st")
            nc.gpsimd.dma_start(s_sb[:], s_d[:])
            rt_sb = wts.tile([128, 128], f32r, tag="rt", name="rt")
            nc.gpsimd.dma_start(rt_sb[:], rt_d[:])
            id_sb = wts.tile([128, D], f32r, tag="id", name="id")
            nc.gpsimd.dma_start(id_sb[:D, :], id_d[:])
            on_sb = wts.tile([1, D], f32, tag="on", name="on")
            nc.gpsimd.dma_start(on_sb[:], on_d[:])
            wo_sb = []
            for i in range(2):
                t = wts.tile([128, HIDDEN], bf16, tag=f"wo{i}", name="wo")
                wo_sb.append(t)
            mk_sb = []
            for j in range(4):
                t = wts.tile([128, 512], bf16, tag=f"mk{j}", name="mk")
                mk_sb.append(t)

            # ---- persistent activations (RoPE applied in place) ----
            qr_sb = [acts.tile([128, S], f32r, tag=f"qr{p}", name=f"qr{p}")
                     for p in range(2)]
            kr_sb = acts.tile([128, S], f32r, tag="kr", name="kr")
            kzA = acts.tile([128, S], f32r, tag="kzA", name="kzA")
            kzB = acts.tile([128, S], f32r, tag="kzB", name="kzB")
            vT_sb = acts.tile([128, S], f32r, tag="vT", name="vT")  # rows 0:64
            v_sb = acts.tile([128, ST, D + 1], bf16, tag="v", name="v")
            outT = [acts.tile([128, S], bf16, tag=f"oT{p}", name=f"oT{p}")
                    for p in range(2)]

            nc.sync.dma_start(v_sb[:, :, D:D + 1],
                              oc_d[:].rearrange("p (t u) -> p t u", u=1))
            nc.sync.dma_start(kzA[D:128, :], zr_d[:])
            nc.sync.dma_start(kzB[0:D, :], zr_d[:])

            # ---- phases A-C fused, by s-quarters: qkv matmuls, psum
            # evacuation (ScalarE), RoPE (PE rot-matmul + DVE), v transpose —
            # keeps the PE dense so HAM stays at full clock ----
            with (
                tc.tile_pool(name="psA", bufs=6, space="PSUM") as psA,
                tc.tile_pool(name="psB", bufs=1, space="PSUM") as psB,
                tc.tile_pool(name="psC", bufs=1, space="PSUM") as psC,
            ):
                for q4 in range(2):
                    sl2 = slice(q4 * 1024, (q4 + 1) * 1024)
                    ps = [[psA.tile([128, 512], f32, tag="qkv",
                                    name=f"qkv{o}{cc}") for cc in range(2)]
                          for o in range(3)]
                    for k in range(KT):
                        xt = xin.tile([128, 1024], f32r, tag="xt", name="xt")
                        nc.sync.dma_start(xt[:], xT_t[k][:, sl2])
                        for o in range(3):
                            for cc in range(2):
                                nc.tensor.matmul(
                                    ps[o][cc][:],
                                    lhsT=wq_sb[k][:, o * 128:(o + 1) * 128],
                                    rhs=xt[:, cc * 512:(cc + 1) * 512],
                                    start=(k == 0), stop=(k == KT - 1))
                    for cc in range(2):
                        sl = slice(q4 * 1024 + cc * 512,
                                   q4 * 1024 + cc * 512 + 512)
                        for o in range(2):
                            nc.scalar.copy(qr_sb[o][:, sl], ps[o][cc][:])
                        nc.scalar.copy(kr_sb[0:D, sl], ps[2][cc][0:D, :])
                        nc.scalar.copy(kr_sb[D:128, sl], ps[2][cc][0:D, :])
                        nc.scalar.copy(vT_sb[0:D, sl], ps[2][cc][D:128, :])
                    # RoPE in place:  t = t*C + (R @ t)*S
                    for cc in range(2):
                        sl = slice(q4 * 1024 + cc * 512,
                                   q4 * 1024 + cc * 512 + 512)
                        for src_t in (qr_sb[0], qr_sb[1], kr_sb):
                            sw = psB.tile([128, 512], f32, tag="sw", name="sw")
                            nc.tensor.matmul(sw[:], lhsT=rt_sb[:],
                                             rhs=src_t[:, sl],
                                             start=True, stop=True)
                            m1 = evp.tile([128, 512], f32, tag="m1", name="m1")
                            nc.vector.tensor_mul(m1[:], src_t[:, sl],
                                                 c_sb[:, sl])
                            m2 = evp.tile([128, 512], f32, tag="m2", name="m2")
                            nc.vector.tensor_mul(m2[:], sw[:], s_sb[:, sl])
                            if src_t is kr_sb:
                                # k: split RoPE'd halves into the zero-padded
                                # K=128 score operands
                                nc.vector.tensor_add(kzA[0:D, sl], m1[0:D, :],
                                                     m2[0:D, :])
                                nc.vector.tensor_add(kzB[D:128, sl],
                                                     m1[D:128, :],
                                                     m2[D:128, :])
                            else:
                                nc.vector.tensor_add(src_t[:, sl], m1[:],
                                                     m2[:])
                    # v transpose for this half's eight st tiles
                    for t in range(8 * q4, 8 * q4 + 8):
                        pv = psC.tile([128, D], f32r, tag="vt", name="vt")
                        nc.tensor.transpose(
                            pv[:], vT_sb[0:D, t * 128:(t + 1) * 128],
                            id_sb[:D, :D])
                        nc.vector.tensor_copy(v_sb[:, t, 0:D], pv[:])

            for i in range(2):
                nc.gpsimd.dma_start(wo_sb[i][:], wo_d[i * 128:(i + 1) * 128, :])
            for j in range(4):
                nc.gpsimd.dma_start(mk_sb[j][:], mk_d[j])

            # ---- phase D/E: flash attention with interleaved projection ----
            with (
                tc.tile_pool(name="scp", bufs=2, space="PSUM") as scp,
                tc.tile_pool(name="avp", bufs=2, space="PSUM") as avp,
                tc.tile_pool(name="mpp", bufs=2, space="PSUM") as mpp,
            ):
                def flash_head(c, h, uo_sb, l_sb):
                    csl = slice(c * 512, (c + 1) * 512)
                    n_st = 4 * c + 4
                    pair, half = divmod(h, 2)
                    kz = kzA if half == 0 else kzB
                    av = avp.tile([128, 512], f32, tag="av", name="av")

                    def av_group(g, pt):
                        for i in range(2):
                            t = 2 * g + i
                            psl = slice(i * 512, (i + 1) * 512)
                            if t >= 4 * c:
                                nc.vector.tensor_mul(
                                    pt[:, psl], pt[:, psl],
                                    mk_sb[t - 4 * c][:])
                            nc.tensor.matmul(
                                av[:D + 1, :],
                                lhsT=v_sb[:, t, :],
                                rhs=pt[:, psl],
                                start=(t == 0), stop=(t == n_st - 1))

                    prev = None
                    for g in range(n_st // 2):
                        sc = scp.tile([128, 1024], f32, tag="sc", name="sc")
                        for i in range(2):
                            t = 2 * g + i
                            nc.tensor.matmul(
                                sc[:, i * 512:(i + 1) * 512],
                                lhsT=kz[:, t * 128:(t + 1) * 128],
                                rhs=qr_sb[pair][:, csl],
                                start=True, stop=True)
                        pt = psb.tile([128, 1024], bf16, tag="P", name="P")
                        nc.scalar.activation(
                            pt[:], sc[:],
                            mybir.ActivationFunctionType.Exp, scale=scale)
                        # emit the PREVIOUS group's attn@V after this group's
                        # scores so the PE never head-of-line blocks on exp
                        if prev is not None:
                            av_group(*prev)
                        prev = (g, pt)
                    av_group(*prev)
                    # stash unnormalized out^T and its denominator row
                    nc.vector.tensor_copy(uo_sb[0:D, :], av[0:D, :])
                    nc.vector.tensor_copy(l_sb[32 * h:32 * h + 1, :],
                                          av[D:D + 1, :])

                def norm_head(c, h, uo_sb, rcp):
                    csl = slice(c * 512, (c + 1) * 512)
                    pair, half = divmod(h, 2)
                    qsl = slice(half * D, (half + 1) * D)
                    bc = mpp.tile([128, 512], f32, tag="mp", name="mp")
                    nc.tensor.matmul(bc[:D, :], lhsT=on_sb[:],
                                     rhs=rcp[h][:], start=True, stop=True)
                    rsb = evp.tile([128, 512], f32, tag="rsb", name="rsb")
                    nc.vector.tensor_copy(rsb[:D, :], bc[:D, :])
                    nc.vector.tensor_mul(
                        outT[pair][qsl, csl], uo_sb[0:D, :], rsb[:D, :])

                def proj_group(c, hts):
                    csl = slice(c * 512, (c + 1) * 512)
                    for ht in hts:
                        pr = mpp.tile([128, 512], f32, tag="mp", name="mp")
                        for i in range(2):
                            nc.tensor.matmul(
                                pr[:],
                                lhsT=wo_sb[i][:, ht * 128:(ht + 1) * 128],
                                rhs=outT[i][:, csl],
                                start=(i == 0), stop=(i == 1))
                        ev = evo.tile([128, 512], f32, tag="ev", name="ev")
                        if ht % 2 == 0:
                            nc.vector.tensor_copy(ev[:], pr[:])
                        else:
                            nc.scalar.copy(ev[:], pr[:])
                        nc.sync.dma_start(
                            out_d[ht * 128:(ht + 1) * 128, csl], ev[:])

                # pipeline: flash(ci) | normalize(ci-1) | project(ci-2) —
                # proj must trail normalization of ALL heads of its chunk
                cs = [0, 1, 2, 3]
                uo_tiles = {}
                rcps = {}
                for i in range(NC4 + 2):
                    if i < NC4:
                        l_sb = evp.tile([128, 512], f32, tag="l", name="l")
                    for h in range(HPC):
                        if i < NC4:
                            uo = evp.tile([128, 512], f32, tag=f"uo{h}",
                                          name=f"uo{h}")
                            uo_tiles[(cs[i], h)] = uo
                            flash_head(cs[i], h, uo, l_sb)
                        if 1 <= i <= NC4:
                            norm_head(cs[i - 1], h,
                                      uo_tiles.pop((cs[i - 1], h)),
                                      rcps[cs[i - 1]])
                        if i >= 2:
                            proj_group(cs[i - 2], range(4 * h, 4 * h + 4))
                    if i < NC4:
                        rcp = evp.tile([128, 512], f32, tag="rcp", name="rcp")
                        nc.vector.reciprocal(rcp[:97, :], l_sb[:97, :])
                        rows = []
                        for h in range(HPC):
                            rh = evp.tile([1, 512], f32, tag=f"rch{h}",
                                          name=f"rch{h}")
                            nc.vector.tensor_copy(rh[:], rcp[32 * h:32 * h + 1, :])
                            rows.append(rh)
                        rcps[cs[i]] = rows

    _split_excess_waits(nc, mybir)
    return nc


def _host_prep(x, cos, sin, w_qkv, w_out):
    import ml_dtypes
    bf = ml_dtypes.bfloat16

    xT = np.ascontiguousarray(x[0].T.astype(np.float32))                # [H, S]
    cosT = cos.T.astype(np.float32)                             # [64, S]
    sinT = sin.T.astype(np.float32)
    ctile = np.ascontiguousarray(np.concatenate([cosT, cosT], 0))
    stile = np.ascontiguousarray(np.concatenate([sinT, sinT], 0))

    # rotate_half as a matrix: rot(q)^T = R @ q^T per 64-block; ship R^T
    r = np.zeros((D, D), dtype=np.float32)
    for i in range(32):
        r[i, 32 + i] = -1.0
        r[32 + i, i] = 1.0
    R = np.zeros((128, 128), dtype=np.float32)
    R[:D, :D] = r
    R[D:, D:] = r
    rotT = np.ascontiguousarray(R.T)

    ident = np.eye(D, dtype=np.float32)
    ones = np.ones((1, D), dtype=np.float32)

    p = np.arange(128)[:, None]
    f = np.arange(512)[None, :]
    masks = np.stack([(p <= f - 128 * j).astype(bf) for j in range(4)])

    shared = {"xT": xT, "ctile": ctile, "stile": stile, "rotT": rotT,
              "ident": ident, "ones": ones, "masks": masks,
              "onecol": np.ones((128, ST), dtype=bf),
              "zeros": np.zeros((D, S), dtype=np.float32)}

    in_maps = []
    for c in range(N_CORES):
        qrows = w_qkv[4 * c * D:(4 * c + 4) * D]                # [256, H]
        krows = w_qkv[N_HEADS * D + c * D: N_HEADS * D + (c + 1) * D]
        vrows = w_qkv[(N_HEADS + N_KV_HEADS) * D + c * D:
                      (N_HEADS + N_KV_HEADS) * D + (c + 1) * D]
        wsh = np.concatenate([qrows, krows, vrows], 0)          # [384, H]
        wqkvT = np.ascontiguousarray(wsh.T.astype(np.float32))  # [H, 384]
        wo_cols = w_out[:, 4 * c * D:(4 * c + 4) * D]           # [H, 256]
        woutT = np.ascontiguousarray(wo_cols.T).astype(bf)
        in_maps.append({**shared, "wqkvT": wqkvT, "woutT": woutT})
    return in_maps


def kernel(x, cos, sin, w_qkv, w_out):
    from concourse.bass_utils import run_bass_kernel_spmd

    if "nc" not in _CACHE:
        _CACHE["nc"] = _build()
    nc = _CACHE["nc"]

    in_maps = _host_prep(x, cos, sin, w_qkv, w_out)
    res = run_bass_kernel_spmd(nc, in_maps, list(range(N_CORES)))
    total = np.zeros((HIDDEN, S), dtype=np.float32)
    for r in res.results:
        total += r["out"].astype(np.float32)
    return total.T.reshape(1, S, HIDDEN).copy()



# revision 14
# speedup vs baseline: 1.2524x; 1.2524x over previous
"""GQA attention (RoPE, causal) on 8 Trainium2 NeuronCores, tensor-parallel
over heads: each core owns 4 query heads + 1 kv head, computes its slice of
qkv, attention, and a partial output projection; the host sums the 8 partial
projections.

All matmuls and the bulk dataflow run in bf16 (fp32 PSUM accumulation), so
x/weights stream at half the HBM bytes and DVE elementwise ops hit the
2x/4x 16-bit modes. Scores are computed transposed ([st, sq]) with
64-partition contraction (no zero-padded K operands), so the softmax
denominator comes out of the attn@V matmul itself via a ones-column
appended to V (M=65), and exp needs no max-subtraction (logits are bounded;
fp32 PSUM can't overflow). Normalization uses reciprocal_approx_fast plus a
GpSimd partition_broadcast (no PE broadcast matmuls). The projection of
chunk c-1 is interleaved into the flash loop of chunk c so the PE has work
while the ScalarE runs exp. Phase A (qkv+RoPE) is software-pipelined in
four 512-column chunks so PSUM evacuation overlaps the next chunk's
matmuls.
"""

import numpy as np

HIDDEN = 2048
HEAD_DIM = 64
N_HEADS = 32
N_KV_HEADS = 8
S = 2048
N_CORES = 8
HPC = N_HEADS // N_CORES          # q heads per core = 4
D = HEAD_DIM
KT = HIDDEN // 128                # 16 contraction tiles for qkv
ST = S // 128                     # 16 seq tiles of 128
NC4 = S // 512                    # 4 seq chunks of 512
OSH = HPC * D + 2 * D             # 384 rows in the per-core qkv weight shard

_CACHE = {}


def _split_excess_waits(nc, mybir):
    """The staged walrus accepts at most one sync wait per instruction (two
    on EventSemaphore); Tile attaches more. Hoist extras onto same-engine
    NoOps inserted just before the instruction — engine program order then
    preserves the wait semantics."""
    for func in nc.m.functions:
        for block in func.blocks:
            new_insts = []
            for inst in block.instructions:
                si = inst.sync_info
                waits = list(si.on_wait) if si is not None and si.on_wait else []
                cap = 2 if isinstance(inst, mybir.InstEventSemaphore) else 1
                if len(waits) > cap:
                    si.on_wait = waits[:cap]
                    for j, w in enumerate(waits[cap:]):
                        nop = mybir.InstNoOp(
                            name=f"{inst.name}-ws{j}",
                            ins=[], outs=[], engine=inst.engine,
                        )
                        nop.sync_info = mybir.SyncInfo(on_wait=[w], on_update=[])
                        new_insts.append(nop)
                new_insts.append(inst)
            block.instructions = new_insts


def _build():
    import concourse.bass as bass
    import concourse.tile as tile
    from concourse import mybir

    f32 = mybir.dt.float32
    f32r = mybir.dt.float32r
    bf16 = mybir.dt.bfloat16

    nc = bass.Bass("TRN2", target_bir_lowering=False, debug=False,
                   num_devices=N_CORES)

    xT_d = nc.dram_tensor("xT", [HIDDEN, S], bf16, kind="ExternalInput")
    wq_d = nc.dram_tensor("wqkvT", [HIDDEN, OSH], bf16, kind="ExternalInput")
    wo_d = nc.dram_tensor("woutT", [2 * 128, HIDDEN], bf16, kind="ExternalInput")
    c_d = nc.dram_tensor("ctile", [128, S], bf16, kind="ExternalInput")
    s_d = nc.dram_tensor("stile", [128, S], bf16, kind="ExternalInput")
    rt_d = nc.dram_tensor("rotT", [128, 128], bf16, kind="ExternalInput")
    id_d = nc.dram_tensor("ident", [D, D], bf16, kind="ExternalInput")
    on_d = nc.dram_tensor("ones", [1, D], f32r, kind="ExternalInput")
    mk_d = nc.dram_tensor("masks", [4, 128, 512], bf16, kind="ExternalInput")
    out_d = nc.dram_tensor("out", [HIDDEN, S], bf16, kind="ExternalOutput")

    xT_t = xT_d.rearrange("(t p) s -> t p s", p=128)
    wq_t = wq_d.rearrange("(t p) o -> t p o", p=128)

    scale = 1.0 / float(np.sqrt(D))

    with tile.TileContext(nc) as tc:
        with (
            nc.allow_low_precision(reason="bf16 dataflow is deliberate"),
            tc.tile_pool(name="wts", bufs=1) as wts,
            tc.tile_pool(name="acts", bufs=1) as acts,
            tc.tile_pool(name="xin", bufs=8) as xin,
            tc.tile_pool(name="psb", bufs=4) as psb,
            tc.tile_pool(name="ev", bufs=2) as evp,
            tc.tile_pool(name="evo", bufs=4) as evo,
        ):
            # ---- persistent loads (gpsimd queue, so the sync queue is
            # free for the xT stream; flash-only tensors loaded later) ----
            wq_sb = []
            for k in range(KT):
                t = wts.tile([128, OSH], bf16, tag=f"wq{k}", name="wq")
                nc.gpsimd.dma_start(t[:], wq_t[k])
                wq_sb.append(t)
            c_sb = wts.tile([128, S], bf16, tag="ct", name="ct")
            nc.gpsimd.dma_start(c_sb[:], c_d[:])
            s_sb = wts.tile([128, S], bf16, tag="st", name="st")
            nc.gpsimd.dma_start(s_sb[:], s_d[:])
            rt_sb = wts.tile([128, 128], bf16, tag="rt", name="rt")
            nc.gpsimd.dma_start(rt_sb[:], rt_d[:])
            id_sb = wts.tile([D, D], bf16, tag="id", name="id")
            nc.gpsimd.dma_start(id_sb[:], id_d[:])
            on_sb = wts.tile([1, D], f32r, tag="on", name="on")
            nc.gpsimd.dma_start(on_sb[:], on_d[:])
            wo_sb = []
            for i in range(2):
                t = wts.tile([128, HIDDEN], bf16, tag=f"wo{i}", name="wo")
                wo_sb.append(t)
            mk_sb = []
            for j in range(4):
                t = wts.tile([128, 512], bf16, tag=f"mk{j}", name="mk")
                mk_sb.append(t)

            # ---- persistent activations (RoPE applied in place) ----
            qr_sb = [acts.tile([128, S], bf16, tag=f"qr{p}", name=f"qr{p}")
                     for p in range(2)]
            kr_sb = acts.tile([128, S], bf16, tag="kr", name="kr")
            vT_sb = acts.tile([D, S], bf16, tag="vT", name="vT")
            v_sb = acts.tile([128, ST, D + 1], bf16, tag="v", name="v")
            outT = [acts.tile([128, S], bf16, tag=f"oT{p}", name=f"oT{p}")
                    for p in range(2)]

            nc.gpsimd.memset(v_sb[:, :, D:D + 1], 1.0)

            # ---- phases A-C, software-pipelined by 512-col chunks:
            # qkv matmuls of chunk j overlap psum evacuation (ScalarE),
            # RoPE (PE rot-matmul + DVE) and v transpose of chunk j-1 ----
            with (
                tc.tile_pool(name="psA", bufs=6, space="PSUM") as psA,
                tc.tile_pool(name="psB", bufs=1, space="PSUM") as psB,
                tc.tile_pool(name="psC", bufs=1, space="PSUM") as psC,
            ):
                ps_chunks = {}

                def qkv_chunk(ch):
                    sl = slice(ch * 512, (ch + 1) * 512)
                    ps = [psA.tile([128, 512], f32, tag="qkv",
                                   name=f"qkv{o}c{ch}") for o in range(3)]
                    ps_chunks[ch] = ps
                    for k in range(KT):
                        xt = xin.tile([128, 512], bf16, tag="xt", name="xt")
                        nc.sync.dma_start(xt[:], xT_t[k][:, sl])
                        for o in range(3):
                            nc.tensor.matmul(
                                ps[o][:],
                                lhsT=wq_sb[k][:, o * 128:(o + 1) * 128],
                                rhs=xt[:],
                                start=(k == 0), stop=(k == KT - 1))

                def finish_chunk(ch):
                    sl = slice(ch * 512, (ch + 1) * 512)
                    ps = ps_chunks.pop(ch)
                    nc.scalar.copy(qr_sb[0][:, sl], ps[0][:])
                    nc.scalar.copy(qr_sb[1][:, sl], ps[1][:])
                    # k duplicated into both partition halves so the flash
                    # score matmuls can contract 64 partitions at base 0
                    # (even heads) or base 64 (odd heads) — matmul requires
                    # lhsT/rhs base partitions to match
                    nc.scalar.copy(kr_sb[0:D, sl], ps[2][0:D, :])
                    nc.scalar.copy(kr_sb[D:128, sl], ps[2][0:D, :])
                    nc.scalar.copy(vT_sb[:, sl], ps[2][D:128, :])
                    # RoPE in place:  t = t*C + (R @ t)*S
                    for src_t in (qr_sb[0], qr_sb[1], kr_sb):
                        p = src_t.shape[0]
                        sw = psB.tile([128, 512], f32, tag="sw", name="sw")
                        nc.tensor.matmul(sw[:p, :], lhsT=rt_sb[:p, :p],
                                         rhs=src_t[:, sl],
                                         start=True, stop=True)
                        m1 = evp.tile([p, 512], bf16, tag="m1", name="m1")
                        nc.vector.tensor_mul(m1[:], src_t[:, sl],
                                             c_sb[:p, sl])
                        m2 = evp.tile([p, 512], bf16, tag="m2", name="m2")
                        nc.vector.tensor_mul(m2[:], sw[:p, :], s_sb[:p, sl])
                        nc.vector.tensor_add(src_t[:, sl], m1[:], m2[:])
                    # v transpose for this chunk's four st tiles, batched
                    # into one psum tile -> one DVE evacuation
                    pv = psC.tile([128, 4 * D], bf16, tag="vt", name="vt")
                    for j in range(4):
                        t = 4 * ch + j
                        nc.tensor.transpose(
                            pv[:, j * D:(j + 1) * D],
                            vT_sb[:, t * 128:(t + 1) * 128],
                            id_sb[:])
                    nc.vector.tensor_copy(
                        v_sb[:, 4 * ch:4 * ch + 4, 0:D],
                        pv[:].rearrange("p (t d) -> p t d", d=D))

                for ch in range(NC4 + 1):
                    if ch < NC4:
                        qkv_chunk(ch)
                    if ch >= 1:
                        finish_chunk(ch - 1)

            for i in range(2):
                nc.gpsimd.dma_start(wo_sb[i][:], wo_d[i * 128:(i + 1) * 128, :])
            for j in range(4):
                nc.gpsimd.dma_start(mk_sb[j][:], mk_d[j])

            # ---- phase D/E: flash attention with interleaved projection ----
            with (
                tc.tile_pool(name="scp", bufs=2, space="PSUM") as scp,
                tc.tile_pool(name="avp", bufs=2, space="PSUM") as avp,
                tc.tile_pool(name="mpp", bufs=2, space="PSUM") as mpp,
            ):
                def flash_head(c, h, uo_sb, l_sb):
                    csl = slice(c * 512, (c + 1) * 512)
                    n_st = 4 * c + 4
                    pair, half = divmod(h, 2)
                    qsl = slice(half * D, (half + 1) * D)
                    av = avp.tile([128, 512], f32, tag="av", name="av")

                    def av_group(g, pt):
                        for i in range(2):
                            t = 2 * g + i
                            psl = slice(i * 512, (i + 1) * 512)
                            if t >= 4 * c:
                                nc.vector.tensor_mul(
                                    pt[:, psl], pt[:, psl],
                                    mk_sb[t - 4 * c][:])
                            nc.tensor.matmul(
                                av[:D + 1, :],
                                lhsT=v_sb[:, t, :],
                                rhs=pt[:, psl],
                                start=(t == 0), stop=(t == n_st - 1))

                    prev = None
                    for g in range(n_st // 2):
                        sc = scp.tile([128, 1024], f32, tag="sc", name="sc")
                        for i in range(2):
                            t = 2 * g + i
                            nc.tensor.matmul(
                                sc[:, i * 512:(i + 1) * 512],
                                lhsT=kr_sb[qsl, t * 128:(t + 1) * 128],
                                rhs=qr_sb[pair][qsl, csl],
                                start=True, stop=True)
                        pt = psb.tile([128, 1024], bf16, tag="P", name="P")
                        nc.scalar.activation(
                            pt[:], sc[:],
                            mybir.ActivationFunctionType.Exp, scale=scale)
                        # emit the PREVIOUS group's attn@V after this group's
                        # scores so the PE never head-of-line blocks on exp
                        if prev is not None:
                            av_group(*prev)
                        prev = (g, pt)
                    av_group(*prev)
                    # stash unnormalized out^T and its denominator row
                    nc.vector.tensor_copy(uo_sb[:], av[0:D, :])
                    nc.vector.tensor_copy(l_sb[32 * h:32 * h + 1, :],
                                          av[D:D + 1, :])

                def norm_head(c, h, uo_sb, rcp):
                    csl = slice(c * 512, (c + 1) * 512)
                    pair, half = divmod(h, 2)
                    qsl = slice(half * D, (half + 1) * D)
                    # broadcast 1/l across the 64 head dims via a ones-column
                    # matmul (f32r rhs -> full-rate), then scale in one mul
                    bc = mpp.tile([128, 512], f32, tag="mp", name="mp")
                    nc.tensor.matmul(bc[:D, :], lhsT=on_sb[:],
                                     rhs=rcp[h][:], start=True, stop=True)
                    nc.vector.tensor_mul(
                        outT[pair][qsl, csl], uo_sb[:], bc[:D, :])

                def proj_group(c, hts):
                    csl = slice(c * 512, (c + 1) * 512)
                    for ht in hts:
                        pr = mpp.tile([128, 512], f32, tag="mp", name="mp")
                        for i in range(2):
                            nc.tensor.matmul(
                                pr[:],
                                lhsT=wo_sb[i][:, ht * 128:(ht + 1) * 128],
                                rhs=outT[i][:, csl],
                                start=(i == 0), stop=(i == 1))
                        ev = evo.tile([128, 512], bf16, tag="ev", name="ev")
                        if ht % 4 == 3:
                            nc.scalar.copy(ev[:], pr[:])
                        else:
                            nc.vector.tensor_copy(ev[:], pr[:])
                        nc.sync.dma_start(
                            out_d[ht * 128:(ht + 1) * 128, csl], ev[:])

                # pipeline: flash(ci) | normalize(ci-1) | project(ci-2) —
                # proj must trail normalization of ALL heads of its chunk
                cs = [0, 1, 2, 3]
                uo_tiles = {}
                rcps = {}
                for i in range(NC4 + 2):
                    if i < NC4:
                        l_sb = evp.tile([128, 512], f32, tag="l", name="l")
                    for h in range(HPC):
                        if i < NC4:
                            uo = evp.tile([D, 512], bf16, tag=f"uo{h}",
                                          name=f"uo{h}")
                            uo_tiles[(cs[i], h)] = uo
                            flash_head(cs[i], h, uo, l_sb)
                        if 1 <= i <= NC4:
                            norm_head(cs[i - 1], h,
                                      uo_tiles.pop((cs[i - 1], h)),
                                      rcps[cs[i - 1]])
                        if i >= 2:
                            proj_group(cs[i - 2], range(4 * h, 4 * h + 4))
                    if i < NC4:
                        # 1/l as exp(-ln l) on ScalarE: ln and exp share one
                        # act table, and it offloads the reciprocal from the
                        # busier DVE
                        lnl = evp.tile([128, 512], f32, tag="lnl", name="lnl")
                        nc.scalar.activation(
                            lnl[:97, :], l_sb[:97, :],
                            mybir.ActivationFunctionType.Ln)
                        rcp = evp.tile([128, 512], f32, tag="rcp", name="rcp")
                        nc.scalar.activation(
                            rcp[:97, :], lnl[:97, :],
                            mybir.ActivationFunctionType.Exp, scale=-1.0)
                        rows = []
                        for h in range(HPC):
                            rh = evp.tile([1, 512], f32r, tag=f"rch{h}",
                                          name=f"rch{h}")
                            nc.vector.tensor_copy(rh[:],
                                                  rcp[32 * h:32 * h + 1, :])
                            rows.append(rh)
                        rcps[cs[i]] = rows

    _split_excess_waits(nc, mybir)
    return nc


def _host_prep(x, cos, sin, w_qkv, w_out):
    import ml_dtypes
    bf = ml_dtypes.bfloat16

    xT = np.ascontiguousarray(x[0].T).astype(bf)                # [H, S]
    cosT = cos.T.astype(np.float32)                             # [64, S]
    sinT = sin.T.astype(np.float32)
    ctile = np.ascontiguousarray(np.concatenate([cosT, cosT], 0)).astype(bf)
    stile = np.ascontiguousarray(np.concatenate([sinT, sinT], 0)).astype(bf)

    # rotate_half as a matrix: rot(q)^T = R @ q^T per 64-block; ship R^T
    r = np.zeros((D, D), dtype=np.float32)
    for i in range(32):
        r[i, 32 + i] = -1.0
        r[32 + i, i] = 1.0
    R = np.zeros((128, 128), dtype=np.float32)
    R[:D, :D] = r
    R[D:, D:] = r
    rotT = np.ascontiguousarray(R.T).astype(bf)

    ident = np.eye(D, dtype=np.float32).astype(bf)

    p = np.arange(128)[:, None]
    f = np.arange(512)[None, :]
    masks = np.stack([(p <= f - 128 * j).astype(bf) for j in range(4)])

    shared = {"xT": xT, "ctile": ctile, "stile": stile, "rotT": rotT,
              "ident": ident, "ones": np.ones((1, D), dtype=np.float32),
              "masks": masks}

    in_maps = []
    for c in range(N_CORES):
        qrows = w_qkv[4 * c * D:(4 * c + 4) * D]                # [256, H]
        krows = w_qkv[N_HEADS * D + c * D: N_HEADS * D + (c + 1) * D]
        vrows = w_qkv[(N_HEADS + N_KV_HEADS) * D + c * D:
                      (N_HEADS + N_KV_HEADS) * D + (c + 1) * D]
        wsh = np.concatenate([qrows, krows, vrows], 0)          # [384, H]
        wqkvT = np.ascontiguousarray(wsh.T).astype(bf)          # [H, 384]
        wo_cols = w_out[:, 4 * c * D:(4 * c + 4) * D]           # [H, 256]
        woutT = np.ascontiguousarray(wo_cols.T).astype(bf)
        in_maps.append({**shared, "wqkvT": wqkvT, "woutT": woutT})
    return in_maps


def kernel(x, cos, sin, w_qkv, w_out):
    from concourse.bass_utils import run_bass_kernel_spmd

    if "nc" not in _CACHE:
        _CACHE["nc"] = _build()
    nc = _CACHE["nc"]

    in_maps = _host_prep(x, cos, sin, w_qkv, w_out)
    res = run_bass_kernel_spmd(nc, in_maps, list(range(N_CORES)))
    total = np.zeros((HIDDEN, S), dtype=np.float32)
    for r in res.results:
        total += r["out"].astype(np.float32)
    return total.T.reshape(1, S, HIDDEN).copy()
